# revision 3
# baseline (speedup 1.0000x reference)
"""EnhancedGraphSAGE on 8 trn2 NeuronCores (Bass/Tile) — v2.

Sharding: 8 graphs per core (batch sorted -> nodes graph-contiguous), each
graph padded to G_slot slots (multiple of 128); padded slots hold h == 0
(enforced by a per-window prefix mask folded into the residual update), so
windows are graph-pure and pooling is exact (h >= 0 so zeros don't break max).

h is kept in bf16: per-core shard hT resident in SBUF (feature-major) plus a
node-major DRAM shard; after the encoder and between SAGE layers the shard is
AllGathered into a replicated [8*S, H] bf16 table. Mean aggregation gathers
h[src] rows per edge with gpsimd.dma_gather striped across 4 SWDGE queues
(queue = src bank); per (group, bank) call the per-chunk one-hot scatter
matrices are built in two batched DVE ops (iota == dlocal) * invdeg using
stride-0 broadcast APs, and the PE accumulates aggT[f, dst] per window in
PSUM as gathered.T @ onehot. hn = agg@Wl + bl + h@Wr, LN + relu + masked
residual in node-major.
"""

import math
import os
from contextlib import ExitStack

import numpy as np
import ml_dtypes

H = 128
HT = 64
NCLS = 8
L = 3
P = 128
NCORES = 8
GPC = 8  # graphs per core
MAX_BANK_ROWS = 32767
NBANKS = 4
GRPW = 8  # windows per gather group (PSUM aggT tiles held at once)
BF16 = ml_dtypes.bfloat16


# ----------------------------------------------------------------------------
# host-side schedule construction
# ----------------------------------------------------------------------------

def _build_schedule(x, edge_index, batch):
    N = x.shape[0]
    E = edge_index.shape[1]
    B = GPC * NCORES
    cnt = np.bincount(batch, minlength=B)
    assert cnt.min() > 0, "empty graph unsupported"
    gstart = np.zeros(B + 1, np.int64)
    np.cumsum(cnt, out=gstart[1:])
    G_slot = int(math.ceil(cnt.max() / P) * P)
    S = GPC * G_slot          # padded slots per core
    W = S // P                # windows per core
    WG = G_slot // P          # windows per graph
    bank_rows = int(math.ceil(NCORES * S / NBANKS))
    assert bank_rows <= MAX_BANK_ROWS

    g_of = batch.astype(np.int64)
    core_of_g = np.arange(B) // GPC
    slot_in_core_base = (np.arange(B) % GPC) * G_slot
    # per-core slot of real node n, and its global replicated position
    slot = slot_in_core_base[g_of] + (np.arange(N) - gstart[g_of])
    p_rep = core_of_g[g_of] * S + slot

    src = edge_index[0].astype(np.int64)
    dst = edge_index[1].astype(np.int64)
    deg = np.bincount(dst, minlength=N).astype(np.float64)
    invdeg_node = 1.0 / np.maximum(deg, 1.0)

    e_core = core_of_g[g_of[dst]]
    e_slot = slot[dst]
    e_psrc = p_rep[src]
    e_inv = invdeg_node[dst]

    e_w = e_slot // P
    e_dl = e_slot % P
    e_bank = e_psrc // bank_rows
    e_idx = e_psrc % bank_rows

    ngroups = (W + GRPW - 1) // GRPW
    assert W % GRPW == 0, (W, GRPW)

    # per (core, window, bank) cell edge lists
    key = ((e_core * W + e_w) * NBANKS + e_bank).astype(np.int64)
    korder = np.argsort(key, kind="stable")
    ks = key[korder]
    bounds = np.searchsorted(ks, np.arange(NCORES * W * NBANKS + 1))

    def cell(c, w, b):
        k = (c * W + w) * NBANKS + b
        return korder[bounds[k]:bounds[k + 1]]

    # Per (group, bank) call: concatenate the group's 8 window cells without
    # per-cell padding. call_chunks = max over cores (pad idx 0 / dl 255).
    # A chunk may straddle window boundaries (per-core different!) -> emit one
    # matmul per (chunk, window) pair present on ANY core; per-core dl data
    # masks the inapplicable edges.
    call_chunks = np.zeros((ngroups, NBANKS), np.int64)
    for g in range(ngroups):
        for b in range(NBANKS):
            m = max(sum(bounds[(c * W + w) * NBANKS + b + 1]
                        - bounds[(c * W + w) * NBANKS + b]
                        for w in range(g * GRPW, (g + 1) * GRPW))
                    for c in range(NCORES))
            call_chunks[g, b] = (m + P - 1) // P

    # matmul (chunk j, window w) pairs per call, union over cores
    mm_lists = {}
    for g in range(ngroups):
        for b in range(NBANKS):
            nch = int(call_chunks[g, b])
            if nch == 0:
                mm_lists[(g, b)] = []
                continue
            pairs = set()
            for c in range(NCORES):
                pos = 0
                for w in range(g * GRPW, (g + 1) * GRPW):
                    n = len(cell(c, w, b))
                    if n == 0:
                        continue
                    j0, j1 = pos // P, (pos + n - 1) // P
                    for j in range(j0, j1 + 1):
                        pairs.add((j, w))
                    pos += n
            mm_lists[(g, b)] = sorted(pairs)
    nmm_total = sum(len(v) for v in mm_lists.values())

    # pack per-core idx (per chunk) and dlocal/invdegE (per matmul) in
    # emission order (each call's idx region 64B-aligned: 32 int16 cols)
    def _acols(nch):
        return -(-int(nch) * P // 16 // 32) * 32

    idx_cols_per_call = [_acols(call_chunks[g, b])
                         for g in range(ngroups) for b in range(NBANKS)]
    total_idx_cols = sum(idx_cols_per_call)
    idx16 = np.zeros((NCORES, 128, total_idx_cols), np.int16)
    onehots = np.zeros((NCORES, P, nmm_total * P), BF16)

    for c in range(NCORES):
        colofs = 0
        mmofs = 0
        for g in range(ngroups):
            for b in range(NBANKS):
                nch = int(call_chunks[g, b])
                if nch == 0:
                    continue
                mml = mm_lists[(g, b)]
                vals = np.zeros(nch * P, np.int64)  # idx (0 = safe pad)
                dls = np.full((len(mml), P), 255.0, np.float32)
                ivs = np.zeros((len(mml), P), np.float32)
                pos = 0
                wofs = {}
                for w in range(g * GRPW, (g + 1) * GRPW):
                    lst = cell(c, w, b)
                    n = len(lst)
                    if n:
                        vals[pos:pos + n] = e_idx[lst]
                        wofs[w] = (pos, lst)
                    pos += n
                for mi, (j, w) in enumerate(mml):
                    if w not in wofs:
                        continue
                    p0, lst = wofs[w]
                    lo = max(p0, j * P)
                    hi = min(p0 + len(lst), (j + 1) * P)
                    if lo >= hi:
                        continue
                    lanes = np.arange(lo, hi) - j * P
                    sel = lst[lo - p0: hi - p0]
                    dls[mi, lanes] = e_dl[sel]
                    ivs[mi, lanes] = e_inv[sel]
                ncols = nch * P // 16
                wrapped = vals.reshape(ncols, 16).T.astype(np.int16)
                for r in range(8):
                    idx16[c, r * 16:(r + 1) * 16, colofs:colofs + ncols] = wrapped
                # oh[p, m*P + n] = (n == dl[m, p]) * iv[m, p]
                oh = (np.arange(P)[None, None, :] == dls[:, :, None]) \
                    * ivs[:, :, None]
                onehots[c, :, mmofs * P:(mmofs + len(mml)) * P] = \
                    oh.transpose(1, 0, 2).reshape(P, len(mml) * P).astype(BF16)
                colofs += _acols(nch)
                mmofs += len(mml)

    # first/last matmul per window for PSUM start/stop: emission order is
    # (bank asc, mm_list order) within the window's group
    win_first = {}
    win_last = {}
    for g in range(ngroups):
        for b in range(NBANKS):
            for mi, (j, w) in enumerate(mm_lists[(g, b)]):
                if w not in win_first:
                    win_first[w] = (b, mi)
                win_last[w] = (b, mi)

    # per-core prefix mask: mask[p, w] = 1 if slot w*P+p is a real node
    prefmask = np.zeros((NCORES, P, W), np.float32)
    for g in range(B):
        c, base = g // GPC, (g % GPC) * G_slot
        for w in range(WG):
            lo = base + w * P
            k = min(max(cnt[g] + base - lo, 0), P)
            prefmask[c, :k, base // P + w] = 1.0

    invcnt = np.zeros((NCORES, GPC), np.float32)
    for g in range(B):
        invcnt[g // GPC, g % GPC] = 1.0 / cnt[g]

    return dict(
        N=N, E=E, B=B, cnt=cnt, gstart=gstart, G_slot=G_slot, S=S, W=W,
        WG=WG, bank_rows=bank_rows, slot=slot, g_of=g_of,
        call_chunks=call_chunks, mm_lists=mm_lists, nmm_total=nmm_total,
        idx16=idx16, onehots=onehots,
        idx_cols_per_call=idx_cols_per_call, total_idx_cols=total_idx_cols,
        win_first=win_first, win_last=win_last, ngroups=ngroups,
        prefmask=prefmask, invcnt=invcnt,
    )


def _host_inputs(sched, x, ts, weights):
    N = sched["N"]
    S, G_slot = sched["S"], sched["G_slot"]
    cnt = sched["cnt"]
    slot = sched["slot"]
    g_of = sched["g_of"]

    xT = np.zeros((NCORES, 4, S), np.float32)  # padded slots stay 0
    for c in range(NCORES):
        sel = (g_of // GPC) == c
        xT[c, :, slot[sel]] = x[sel]

    iota = np.tile(np.arange(P, dtype=np.float32), (P, 1)).astype(BF16)
    ident = np.eye(P, dtype=np.float32).astype(BF16)

    per_core = []
    for c in range(NCORES):
        d = {
            "xT": np.ascontiguousarray(xT[c]),
            "gidx": np.ascontiguousarray(sched["idx16"][c]),
            "onehots": np.ascontiguousarray(sched["onehots"][c]),
            "tsT": np.ascontiguousarray(ts[c * GPC:(c + 1) * GPC].T.astype(np.float32)),
            "invcnt": sched["invcnt"][c:c + 1],
            "prefmask": np.ascontiguousarray(sched["prefmask"][c]),
            "iota": iota,
            "ident": ident,
        }
        for k, v in weights.items():
            d[k] = v
        per_core.append(d)
    return per_core


# ----------------------------------------------------------------------------
# bass program
# ----------------------------------------------------------------------------

def _build_nc(sched):
    import concourse.bacc as bacc
    import concourse.bass as bass
    import concourse.mybir as mybir
    import concourse.tile as tile
    from concourse import library_config

    f32 = mybir.dt.float32
    bf16 = mybir.dt.bfloat16
    AF = mybir.ActivationFunctionType
    OP = mybir.AluOpType

    S, W = sched["S"], sched["W"]
    bank_rows = sched["bank_rows"]
    ngroups = sched["ngroups"]
    call_chunks = sched["call_chunks"]
    mm_lists = sched["mm_lists"]
    nmm_total = sched["nmm_total"]
    total_idx_cols = sched["total_idx_cols"]
    win_first, win_last = sched["win_first"], sched["win_last"]

    stage = os.environ.get("GNN_STAGE", "full")
    flags = set(stage.split("+"))
    NQ = int(os.environ.get("GNN_NQ", "4"))
    GBUFS = int(os.environ.get("GNN_GBUFS", "6"))
    nc = bacc.Bacc("TRN2", target_bir_lowering=False, num_swdge_queues=NQ)

    def din(name, shape, dtype=f32):
        return nc.dram_tensor(name, shape, dtype, kind="ExternalInput")

    xT_d = din("xT", [4, S])
    gidx_d = din("gidx", [128, total_idx_cols], mybir.dt.int16)
    onehots_d = din("onehots", [P, nmm_total * P], bf16)
    tsT_d = din("tsT", [3, GPC])
    invcnt_d = din("invcnt", [1, GPC])
    prefmask_d = din("prefmask", [P, W])
    iota_d = din("iota", [P, P], bf16)
    ident_d = din("ident", [P, P], bf16)
    encW_d = din("enc_W", [4, H])
    encb_d = din("enc_b", [H])
    Wl_d = din("sage_Wl", [L * H, H], bf16)
    bl_d = din("sage_bl", [L, H])
    Wr_d = din("sage_Wr", [L * H, H], bf16)
    lng_d = din("ln_g", [L, H])
    lnb_d = din("ln_b", [L, H])
    tsW1_d = din("ts_W1", [3, HT])
    tsb1_d = din("ts_b1", [HT])
    tslng_d = din("ts_lng", [HT])
    tslnb_d = din("ts_lnb", [HT])
    tsW2_d = din("ts_W2", [HT, HT])
    tsb2_d = din("ts_b2", [HT])
    clng_d = din("cls_lng", [2 * H + HT])
    clnb_d = din("cls_lnb", [2 * H + HT])
    cW1_d = din("cls_W1", [2 * H + HT, H])
    cb1_d = din("cls_b1", [H])
    cW2_d = din("cls_W2", [H, NCLS])
    cb2_d = din("cls_b2", [NCLS])
    out_d = nc.dram_tensor("out", [GPC, NCLS], f32, kind="ExternalOutput")
    DBG = int(os.environ.get("GNN_DEBUG", "0"))
    dbg_d = (nc.dram_tensor("dbg", [(L + 1) * S, H], bf16, kind="ExternalOutput")
             if DBG else None)

    h_shard = [nc.dram_tensor(f"h_shard{l}", [S, H], bf16) for l in range(L)]
    h_rep = [nc.dram_tensor(f"h_rep{l}", [NCORES * S, H], bf16,
                            addr_space="Shared") for l in range(L)]

    def bcast_row(dram_ap, npart, width):
        return bass.AP(tensor=dram_ap.tensor, offset=dram_ap.offset,
                       ap=[[0, npart]] + dram_ap.ap[-1:])

    with tile.TileContext(nc) as tc, ExitStack() as ctx:
        res = ctx.enter_context(tc.tile_pool(name="res", bufs=1))
        gath = ctx.enter_context(tc.tile_pool(name="gath", bufs=GBUFS))
        oh = ctx.enter_context(tc.tile_pool(name="oh", bufs=int(os.environ.get("GNN_OHBUFS", "6"))))
        stg = ctx.enter_context(tc.tile_pool(name="stg", bufs=4))
        sml = ctx.enter_context(tc.tile_pool(name="sml", bufs=2))
        ps_agg = ctx.enter_context(tc.tile_pool(name="ps_agg", bufs=4, space="PSUM"))
        ps_hn = ctx.enter_context(tc.tile_pool(name="ps_hn", bufs=2, space="PSUM"))
        ps_t = ctx.enter_context(tc.tile_pool(name="ps_t", bufs=2, space="PSUM"))

        nc.gpsimd.load_library(library_config.mlp)

        # ---- residents ----
        hT = res.tile([P, S], bf16)                     # feature-major h shard
        hNM = res.tile([P, S], bf16)                    # node-major (window-blocked)
        gidx_s = res.tile([128, total_idx_cols], mybir.dt.int16)
        ident_s = res.tile([P, P], bf16)
        ident32_s = res.tile([P, P], f32)
        pmask_s = res.tile([P, W], f32)
        encW_s = res.tile([4, H], f32)
        encb_r = res.tile([P, H], f32)                  # enc_b bcast down parts
        eps_c = res.tile([P, 1], f32)
        nc.sync.dma_start(gidx_s[:], gidx_d[:])
        nc.sync.dma_start(ident_s[:], ident_d[:])
        nc.vector.tensor_copy(ident32_s[:], ident_s[:])
        nc.sync.dma_start(pmask_s[:], prefmask_d[:])
        nc.sync.dma_start(encW_s[:], encW_d[:])
        nc.sync.dma_start(encb_r[:], bcast_row(encb_d[:], P, H))
        nc.vector.memset(eps_c[:], 1e-5)

        def bcast_chunk(t, c0, nch, rep):
            # [P, nch] column block broadcast: element (p, k) -> t[p, c0 + k//rep]
            a = t[:, c0:c0 + nch]
            return bass.AP(tensor=a.tensor, offset=a.offset,
                           ap=[a.ap[0], a.ap[-1], [0, rep]])

        def dram_fence(dram_t):
            # Order: prior nc.sync DMA writes (SP FIFO) -> this read -> its
            # completion sem -> gpsimd copy -> subsequent Pool-queue collective.
            f1 = sml.tile([1, H], bf16, tag="fence1")
            f2 = sml.tile([1, H], bf16, tag="fence2")
            nc.sync.dma_start(f1[:], dram_t[S - 1:S, :])
            nc.gpsimd.tensor_copy(f2[:], f1[:])

        def _pipeline():
            # ---- encoder: h = relu(x @ enc_W + b) * mask, node-major ----
            for w in range(W):
                sl = slice(w * P, (w + 1) * P)
                xw = stg.tile([4, P], f32, tag="xw")
                nc.sync.dma_start(xw[:], xT_d[:, sl])
                ps = ps_hn.tile([P, H], f32, tag="ph")
                nc.tensor.matmul(ps[:], lhsT=xw[:], rhs=encW_s[:],
                                 start=True, stop=True)
                hb = stg.tile([P, H], f32, tag="hb")
                nc.vector.tensor_tensor(hb[:], ps[:], encb_r[:], OP.add)
                hr = stg.tile([P, H], bf16, tag="hr")
                nc.scalar.activation(hr[:], hb[:], AF.Relu)
                nc.vector.tensor_scalar(hNM[:, sl], hr[:], pmask_s[:, w:w + 1],
                                        None, OP.mult)
                nc.sync.dma_start(h_shard[0][sl, :], hNM[:, sl])
                if DBG:
                    nc.sync.dma_start(dbg_d[sl, :], hNM[:, sl])
                pt = ps_t.tile([P, P], bf16, tag="pt")
                nc.tensor.transpose(pt[:], hNM[:, sl], ident_s[:])
                nc.scalar.activation(hT[:, sl], pt[:], AF.Copy)
            if not flags & {"noag", "nolayers"}:
                dram_fence(h_shard[0])
                nc.gpsimd.collective_compute(
                    "AllGather", OP.bypass, ins=[h_shard[0].ap()],
                    outs=[h_rep[0].ap()], replica_groups=[list(range(NCORES))])

            # ---- SAGE layers ----
            for l in range(L if "nolayers" not in flags else 0):
                Wl_s = sml.tile([H, H], bf16, tag="wl")
                Wr_s = sml.tile([H, H], bf16, tag="wr")
                blb = sml.tile([P, H], f32, tag="blb")
                gb = sml.tile([P, H], f32, tag="gb")
                bb = sml.tile([P, H], f32, tag="bb")
                nc.sync.dma_start(Wl_s[:], Wl_d[l * H:(l + 1) * H, :])
                nc.sync.dma_start(Wr_s[:], Wr_d[l * H:(l + 1) * H, :])
                nc.sync.dma_start(blb[:], bcast_row(bl_d[l, :], P, H))
                nc.sync.dma_start(gb[:], bcast_row(lng_d[l, :], P, H))
                nc.sync.dma_start(bb[:], bcast_row(lnb_d[l, :], P, H))

                colofs = 0
                mmofs = 0
                for g in range(ngroups):
                    gtiles = {}
                    ohtiles = {}
                    mm0 = {}
                    for b in range(NBANKS):
                        nch = int(call_chunks[g, b])
                        if nch == 0:
                            continue
                        mml = mm_lists[(g, b)]
                        nmm = len(mml)
                        ncols = nch * P // 16
                        acols = -(-ncols // 32) * 32
                        if "nogather" not in flags:
                            gt = gath.tile([P, nch, P], bf16, tag="gath")
                            nc.gpsimd.dma_gather(
                                gt[:], h_rep[l][b * bank_rows:(b + 1) * bank_rows, :],
                                gidx_s[:, colofs:colofs + ncols],
                                nch * P, nch * P, H,
                                single_packet=(nch * P <= 1024),
                                queue_num=b % NQ)
                            gtiles[b] = gt
                            oht = oh.tile([P, nmm * P], bf16, tag="oh")
                            nc.sync.dma_start(
                                oht[:],
                                onehots_d[:, mmofs * P:(mmofs + nmm) * P])
                            ohtiles[b] = oht
                        mm0[b] = mmofs
                        colofs += acols
                        mmofs += nmm
                    # process the group's 8 windows in two half-passes so
                    # each aggT gets its own PSUM bank (start=True clears the
                    # whole bank's has_written bits)
                    for half in range(2):
                        wlo = g * GRPW + half * (GRPW // 2)
                        whi = wlo + GRPW // 2
                        psw = {}
                        if not flags & {"nogather", "gatheronly"}:
                            for w in range(wlo, whi):
                                if w in win_first:
                                    psw[w] = ps_agg.tile([P, P], f32, tag="aggw",
                                                         name=f"aggw{w}")
                            for b in range(NBANKS):
                                if b not in gtiles or "gatheronly" in flags:
                                    continue
                                for mi, (j, w) in enumerate(mm_lists[(g, b)]):
                                    if not (wlo <= w < whi):
                                        continue
                                    nc.tensor.matmul(
                                        psw[w][:], lhsT=gtiles[b][:, j, :],
                                        rhs=ohtiles[b][:, mi * P:(mi + 1) * P],
                                        start=(win_first[w] == (b, mi)),
                                        stop=(win_last[w] == (b, mi)))
                        # window tails
                        for w in range(wlo, whi):
                            sl = slice(w * P, (w + 1) * P)
                            aggT = stg.tile([P, P], bf16, tag="aggT")
                            if w in psw:
                                nc.scalar.activation(aggT[:], psw[w][:], AF.Copy)
                            else:
                                nc.vector.memset(aggT[:], 0.0)
                            ph = ps_hn.tile([P, H], f32, tag="ph")
                            nc.tensor.matmul(ph[:], lhsT=aggT[:], rhs=Wl_s[:],
                                             start=True, stop=False)
                            nc.tensor.matmul(ph[:], lhsT=hT[:, sl], rhs=Wr_s[:],
                                             start=False, stop=True)
                            hn = stg.tile([P, H], f32, tag="hn_s")
                            nc.vector.tensor_tensor(hn[:], ph[:], blb[:], OP.add)
                            stats = sml.tile([P, 6], f32, tag="st6")
                            mv = sml.tile([P, 2], f32, tag="mv")
                            nc.vector.bn_stats(stats[:], hn[:])
                            nc.vector.bn_aggr(mv[:], stats[:])
                            rstd = sml.tile([P, 1], f32, tag="rstd")
                            nc.scalar.activation(rstd[:], mv[:, 1:2], AF.Sqrt,
                                                 bias=eps_c[:])
                            nc.vector.reciprocal(rstd[:], rstd[:])
                            t1 = stg.tile([P, H], f32, tag="t1")
                            nc.vector.scalar_tensor_tensor(
                                t1[:], hn[:], mv[:, 0:1], gb[:],
                                OP.subtract, OP.mult)
                            nc.vector.scalar_tensor_tensor(
                                t1[:], t1[:], rstd[:], bb[:], OP.mult, OP.add)
                            t1r = stg.tile([P, H], bf16, tag="t1r")
                            nc.scalar.activation(t1r[:], t1[:], AF.Relu)
                            nc.vector.scalar_tensor_tensor(
                                hNM[:, sl], t1r[:], pmask_s[:, w:w + 1], hNM[:, sl],
                                OP.mult, OP.add)
                            if l < L - 1:
                                nc.sync.dma_start(h_shard[l + 1][sl, :], hNM[:, sl])
                            if DBG:
                                nc.sync.dma_start(
                                    dbg_d[(l + 1) * S + w * P:(l + 1) * S + (w + 1) * P, :],
                                    hNM[:, sl])
                            pt2 = ps_t.tile([P, P], bf16, tag="pt")
                            nc.tensor.transpose(pt2[:], hNM[:, sl], ident_s[:])
                            nc.scalar.activation(hT[:, sl], pt2[:], AF.Copy)
                if l < L - 1 and "noag" not in flags:
                    dram_fence(h_shard[l + 1])
                    nc.gpsimd.collective_compute(
                        "AllGather", OP.bypass, ins=[h_shard[l + 1].ap()],
                        outs=[h_rep[l + 1].ap()],
                        replica_groups=[list(range(NCORES))])

            # ---- pooling (hT holds final h; padded slots are 0, h >= 0) ----
            wsum = res.tile([P, W], f32)
            wmax = res.tile([P, W], f32)
            for w in range(W):
                sl = slice(w * P, (w + 1) * P)
                nc.vector.reduce_sum(wsum[:, w:w + 1], hT[:, sl],
                                     axis=mybir.AxisListType.X)
                nc.vector.reduce_max(wmax[:, w:w + 1], hT[:, sl],
                                     axis=mybir.AxisListType.X)
            WG = sched["WG"]
            gsum = sml.tile([P, GPC], f32, tag="gsum")
            gmax = sml.tile([P, GPC], f32, tag="gmax")
            for g in range(GPC):
                nc.vector.reduce_sum(gsum[:, g:g + 1], wsum[:, g * WG:(g + 1) * WG],
                                     axis=mybir.AxisListType.X)
                nc.vector.reduce_max(gmax[:, g:g + 1], wmax[:, g * WG:(g + 1) * WG],
                                     axis=mybir.AxisListType.X)
            icb = sml.tile([P, GPC], f32, tag="icb")
            nc.sync.dma_start(icb[:], bcast_row(invcnt_d[0, :], P, GPC))
            nc.vector.tensor_tensor(gsum[:], gsum[:], icb[:], OP.mult)

            # ---- trackster encoder (feature-major, GPC graphs) ----
            tsT_s = sml.tile([3, GPC], f32, tag="tsT")
            tsW1_s = sml.tile([3, HT], f32, tag="tsW1")
            tsW2_s = sml.tile([HT, HT], f32, tag="tsW2")
            tsb1_c = sml.tile([HT, 1], f32, tag="tsb1")
            tsb2_c = sml.tile([HT, 1], f32, tag="tsb2")
            nc.sync.dma_start(tsT_s[:], tsT_d[:])
            nc.sync.dma_start(tsW1_s[:], tsW1_d[:])
            nc.sync.dma_start(tsW2_s[:], tsW2_d[:])
            nc.sync.dma_start(tsb1_c[:], tsb1_d[:].rearrange("h -> h ()"))
            nc.sync.dma_start(tsb2_c[:], tsb2_d[:].rearrange("h -> h ()"))
            p1 = ps_hn.tile([HT, GPC], f32, tag="ph")
            nc.tensor.matmul(p1[:], lhsT=tsW1_s[:], rhs=tsT_s[:], start=True, stop=True)
            t1T = sml.tile([HT, GPC], f32, tag="t1T")
            nc.scalar.activation(t1T[:], p1[:], AF.Identity, bias=tsb1_c[:])
            pg = ps_t.tile([GPC, HT], f32, tag="pt")
            nc.tensor.transpose(pg[:], t1T[:], ident32_s[:HT, :HT])
            t1g = sml.tile([GPC, HT], f32, tag="t1g")
            nc.vector.tensor_copy(t1g[:], pg[:])
            tst = sml.tile([GPC, 6], f32, tag="tst6")
            tmv = sml.tile([GPC, 2], f32, tag="tsmv")
            nc.vector.bn_stats(tst[:], t1g[:])
            nc.vector.bn_aggr(tmv[:], tst[:])
            trs = sml.tile([GPC, 1], f32, tag="tsrstd")
            nc.scalar.activation(trs[:], tmv[:, 1:2], AF.Sqrt, bias=eps_c[:GPC, :])
            nc.vector.reciprocal(trs[:], trs[:])
            tlgb = sml.tile([GPC, HT], f32, tag="tlgb")
            tlbb = sml.tile([GPC, HT], f32, tag="tlbb")
            nc.sync.dma_start(tlgb[:], bcast_row(tslng_d[:], GPC, HT))
            nc.sync.dma_start(tlbb[:], bcast_row(tslnb_d[:], GPC, HT))
            nc.vector.scalar_tensor_tensor(t1g[:], t1g[:], tmv[:, 0:1], tlgb[:],
                                           OP.subtract, OP.mult)
            nc.vector.scalar_tensor_tensor(t1g[:], t1g[:], trs[:], tlbb[:],
                                           OP.mult, OP.add)
            nc.scalar.activation(t1g[:], t1g[:], AF.Relu)
            pr = ps_t.tile([HT, GPC], f32, tag="pt")
            nc.tensor.transpose(pr[:], t1g[:], ident32_s[:GPC, :GPC])
            t1nT = sml.tile([HT, GPC], f32, tag="t1nT")
            nc.vector.tensor_copy(t1nT[:], pr[:])
            p2 = ps_hn.tile([HT, GPC], f32, tag="ph")
            nc.tensor.matmul(p2[:], lhsT=tsW2_s[:], rhs=t1nT[:], start=True, stop=True)
            t2T = sml.tile([HT, GPC], f32, tag="t2T")
            nc.scalar.activation(t2T[:], p2[:], AF.Identity, bias=tsb2_c[:])

            # ---- classifier ----
            PD = 2 * H + HT
            feat = sml.tile([GPC, PD], f32, tag="feat")
            pf = ps_t.tile([GPC, P], f32, tag="pt")
            nc.tensor.transpose(pf[:], gsum[:], ident32_s[:])
            nc.vector.tensor_copy(feat[:, 0:H], pf[:])
            pf2 = ps_t.tile([GPC, P], f32, tag="pt")
            nc.tensor.transpose(pf2[:], gmax[:], ident32_s[:])
            nc.vector.tensor_copy(feat[:, H:2 * H], pf2[:])
            pf3 = ps_t.tile([GPC, HT], f32, tag="pt")
            nc.tensor.transpose(pf3[:], t2T[:], ident32_s[:HT, :HT])
            nc.vector.tensor_copy(feat[:, 2 * H:PD], pf3[:])
            cst = sml.tile([GPC, 6], f32, tag="cst")
            cmv = sml.tile([GPC, 2], f32, tag="cmv")
            nc.vector.bn_stats(cst[:], feat[:])
            nc.vector.bn_aggr(cmv[:], cst[:])
            crs = sml.tile([GPC, 1], f32, tag="crs")
            nc.scalar.activation(crs[:], cmv[:, 1:2], AF.Sqrt, bias=eps_c[:GPC, :])
            nc.vector.reciprocal(crs[:], crs[:])
            cgb = sml.tile([GPC, PD], f32, tag="cgb")
            cbb = sml.tile([GPC, PD], f32, tag="cbb")
            nc.sync.dma_start(cgb[:], bcast_row(clng_d[:], GPC, PD))
            nc.sync.dma_start(cbb[:], bcast_row(clnb_d[:], GPC, PD))
            nc.vector.scalar_tensor_tensor(feat[:], feat[:], cmv[:, 0:1], cgb[:],
                                           OP.subtract, OP.mult)
            nc.vector.scalar_tensor_tensor(feat[:], feat[:], crs[:], cbb[:],
                                           OP.mult, OP.add)
            cb1_c = sml.tile([H, 1], f32, tag="cb1")
            nc.sync.dma_start(cb1_c[:], cb1_d[:].rearrange("h -> h ()"))
            pz = ps_hn.tile([H, GPC], f32, tag="ph")
            for j, (a, b_) in enumerate([(0, H), (H, 2 * H), (2 * H, PD)]):
                cW1j = sml.tile([b_ - a, H], f32, tag="cW1j", name=f"cW1j{j}")
                nc.sync.dma_start(cW1j[:], cW1_d[a:b_, :])
                pfj = ps_t.tile([b_ - a, GPC], f32, tag="pt")
                nc.tensor.transpose(pfj[:], feat[:, a:b_],
                                    ident32_s[:GPC, :GPC])
                fTj = sml.tile([b_ - a, GPC], f32, tag="fTj")
                nc.vector.tensor_copy(fTj[:], pfj[:])
                nc.tensor.matmul(pz[:], lhsT=cW1j[:], rhs=fTj[:],
                                 start=(j == 0), stop=(j == 2))
            zT = sml.tile([H, GPC], f32, tag="zT")
            nc.scalar.activation(zT[:], pz[:], AF.Relu, bias=cb1_c[:])
            cW2_s = sml.tile([H, NCLS], f32, tag="cW2")
            nc.sync.dma_start(cW2_s[:], cW2_d[:])
            po = ps_hn.tile([GPC, NCLS], f32, tag="ph")
            nc.tensor.matmul(po[:], lhsT=zT[:], rhs=cW2_s[:], start=True, stop=True)
            ob = sml.tile([GPC, NCLS], f32, tag="ob")
            nc.sync.dma_start(ob[:], bcast_row(cb2_d[:], GPC, NCLS))
            outs = sml.tile([GPC, NCLS], f32, tag="outs")
            nc.vector.tensor_tensor(outs[:], po[:], ob[:], OP.add)
            nc.sync.dma_start(out_d[:], outs[:])

        REPS = int(os.environ.get("GNN_REPS", "1"))
        for _rep in range(REPS):
            _pipeline()

    nc.compile()
    return nc


# ----------------------------------------------------------------------------
# entry point
# ----------------------------------------------------------------------------

def kernel(**inputs):
    from concourse.bass_utils import run_bass_kernel_spmd

    x = np.asarray(inputs["x"], np.float32)
    edge_index = np.asarray(inputs["edge_index"])
    batch = np.asarray(inputs["batch"])
    ts = np.asarray(inputs["ts"], np.float32)

    weights = {
        "enc_W": np.asarray(inputs["enc_W"], np.float32),
        "enc_b": np.asarray(inputs["enc_b"], np.float32),
        "sage_Wl": np.asarray(inputs["sage_Wl"], np.float32).reshape(L * H, H).astype(BF16),
        "sage_bl": np.asarray(inputs["sage_bl"], np.float32),
        "sage_Wr": np.asarray(inputs["sage_Wr"], np.float32).reshape(L * H, H).astype(BF16),
        "ln_g": np.asarray(inputs["ln_g"], np.float32),
        "ln_b": np.asarray(inputs["ln_b"], np.float32),
        "ts_W1": np.asarray(inputs["ts_W1"], np.float32),
        "ts_b1": np.asarray(inputs["ts_b1"], np.float32),
        "ts_lng": np.asarray(inputs["ts_lng"], np.float32),
        "ts_lnb": np.asarray(inputs["ts_lnb"], np.float32),
        "ts_W2": np.asarray(inputs["ts_W2"], np.float32),
        "ts_b2": np.asarray(inputs["ts_b2"], np.float32),
        "cls_lng": np.asarray(inputs["cls_lng"], np.float32),
        "cls_lnb": np.asarray(inputs["cls_lnb"], np.float32),
        "cls_W1": np.asarray(inputs["cls_W1"], np.float32),
        "cls_b1": np.asarray(inputs["cls_b1"], np.float32),
        "cls_W2": np.asarray(inputs["cls_W2"], np.float32),
        "cls_b2": np.asarray(inputs["cls_b2"], np.float32),
    }

    sched = _build_schedule(x, edge_index, batch)
    per_core = _host_inputs(sched, x, ts, weights)
    nc = _build_nc(sched)
    res = run_bass_kernel_spmd(nc, per_core, list(range(NCORES)), **_run_kwargs)
    if _res_hook is not None:
        _res_hook(res)
    return np.concatenate([res.results[c]["out"] for c in range(NCORES)], axis=0)


_run_kwargs = {}
_res_hook = None
